# revision 30
# baseline (speedup 1.0000x reference)
"""Segment-mean-of-means kernel for Trainium2 (8 NeuronCores, SPMD).

Problem: out = mean_s( segment_sum(x)[s] / max(count_s, 1) ) over 65536
segments of a [4M, 64] fp32 tensor with *sorted* segment ids.

Mathematical reformulation: every atom i in segment s contributes
x_i / count_s to the segment mean, so

    out[f] = (1/N0) * sum_s segsum_s[f]/count_s = (1/N0) * sum_i w_i * x_i[f]

with per-row weight w_i = 1 / count_{seg(i)}.  Empty segments contribute
nothing, exactly matching the reference's max(count,1) clamp.

Device kernel = pure streaming row-sum in FP8 (e4m3):
  - host: w folded into x (x' = w*x, scaled by a power-of-2 alpha), then
    *noise-shaped* quantization to e4m3: rows are processed in chains of
    L consecutive rows; each element absorbs the previous element's
    quantization error (error feedback / sigma-delta).  The chain's total
    error telescopes to a single final carry, so the *sum* of the fp8
    stream matches the fp64 sum to ~1e-3 relative even though individual
    elements only carry 3 mantissa bits.  (Plain RTN fp8 fails: 3e-2.)
  - device (per core, 1/8 of rows): PSUM-accumulated PE matmuls with an
    all-ones fp8 lhsT, using COLUMN TILING: the 128x128 PE array is split
    into NT independent 128x32 column tiles, each with its own XBUS ifmap
    stream, so NT matmuls run concurrently -> NT fp8 elements per
    partition-cycle (the single-stream ifmap rate, 1 elem/cycle/partition
    @2.4GHz, is otherwise the bottleneck: 104us/core).
  - host: sum the tiny per-core partials, divide by alpha*N0.

Layout: each core's nloc rows are reshaped [128, J, 64] -- partition k
owns rows [k*J, (k+1)*J), a J*64-byte CONTIGUOUS stream in DRAM.  A DMA
moves SD slots for all partitions (one contiguous SD*64-byte descriptor
per partition; big descriptors keep the HWDGE rings off the critical
path).  The PE consumes 512-byte chunks (8 slots):
  chunk c: lhsT = ones[128, 16] @ column quadrant 32*(c%NT),
           rhs  = chunk [128, 512]
  -> psum[32*(c%NT) .. +16 rows (identical), 0:512]
accumulated over that tile's chunks (start on first, stop on last).
psum column n holds a partial of feature (n % 64); the host folds the
8-slot blocks, quadrants, and cores.
"""

import os

import numpy as np

import concourse.bass as bass
import concourse.mybir as mybir
from concourse import bacc
from concourse.bass_utils import run_bass_kernel_spmd
from concourse.tile import TileContext

import ml_dtypes

E4M3 = np.dtype(ml_dtypes.float8_e4m3fn)


def _harden_trace_path():
    """If a caller enables tracing (e.g. BASS_TRACE=1), run_bass_kernel_spmd
    imports antenv.axon_hooks, which this image lacks -- that would crash the
    run.  Provide the hook via trn_boot's ctypes shim (or a None hook, which
    bass_utils degrades on gracefully), and make the artifact upload failure
    non-fatal (zero-egress sandbox)."""
    import sys
    import types

    try:
        import antenv.axon_hooks  # noqa: F401  # already provided: nothing to do
        return
    except ImportError:
        pass
    hook = None
    try:
        import trn_agent_boot.trn_boot as tb

        hook = tb._ntff_profile_via_ctypes("/opt/axon/libaxon_pjrt.so")
    except Exception:
        pass
    mod = types.ModuleType("antenv.axon_hooks")
    mod.get_axon_ntff_profile_hook = lambda: hook
    sys.modules["antenv.axon_hooks"] = mod

    import concourse.bass_utils as bu

    _orig_upload = bu.upload_artifacts

    def _safe_upload(tmpdir):
        try:
            return _orig_upload(tmpdir)
        except Exception:
            return tmpdir

    bu.upload_artifacts = _safe_upload


_harden_trace_path()

F = 64  # features
NC = 8  # cores
SD = int(os.environ.get("KERNEL_SD", "256"))  # slots per DMA (SD*64 B/partition)
WARM = int(os.environ.get("KERNEL_WARM", "4"))  # small leading DMAs
WARM_SD = int(os.environ.get("KERNEL_WARM_SD", "64"))  # slots per warmup DMA
COOL = int(os.environ.get("KERNEL_COOL", "2"))  # small trailing DMAs
COOL_SD = int(os.environ.get("KERNEL_COOL_SD", "32"))  # slots per cooldown DMA
STAG = int(os.environ.get("KERNEL_STAG", "8"))  # stop-stagger, chunks/quadrant
XBUFS = int(os.environ.get("KERNEL_XBUFS", "10"))  # x tile buffering depth
TWO_Q = os.environ.get("KERNEL_2Q", "1") == "1"  # alternate SP/Act HWDGE rings
SPLIT_DMA = os.environ.get("KERNEL_SPLIT", "0") == "1"  # split partitions across rings
NT = int(os.environ.get("KERNEL_NT", "4"))  # concurrent PE column tiles (1..4)
CHAIN_L = int(os.environ.get("KERNEL_L", "256"))  # noise-shaping chain length
N0_DEFAULT = 65536

CH = 512  # rhs bytes/partition per matmul chunk (psum free = 512 fp32)
CSLOTS = CH // F  # 8 slots per chunk
MO = 16  # lhsT (all-ones) columns -> psum rows, identical; col_grp rounds to 32
COL_OFF = {1: [0], 2: [0, 64], 3: [0, 32, 64], 4: [0, 32, 64, 96]}[NT]
assert SD % CSLOTS == 0

_bass_cache: dict = {}


def _build_bass(J: int) -> bass.Bass:
    """One-core SPMD program: fp8 row-sum of 128*J rows ([128, J*64] layout)."""
    dtype = mybir.dt.float8e4
    nch = J // CSLOTS  # total matmul chunks
    # DMA plan: WARM small tiles first (early PE start), SD-slot tiles, then
    # COOL small tiles (short post-last-data matmul tail)
    plan = []
    pos = 0
    cool_slots = min(COOL * COOL_SD, J)
    while pos < J:
        left = J - pos
        if len(plan) < WARM:
            sd = WARM_SD
        elif left > cool_slots:
            sd = min(SD, left - cool_slots)
        else:
            sd = COOL_SD
        sd = min(sd, left)
        plan.append((pos, sd))
        pos += sd
    # chunk -> column-tile assignment: round-robin, but quadrant j goes
    # quiet STAG*(NT-1-j) chunks before the end so its PSUM evacuation
    # overlaps the remaining stream instead of serializing after it.
    stag = STAG if nch >= NT * STAG * 4 else 0
    nact_of = lambda c: max(
        1, sum(1 for j in range(NT) if c < nch - (NT - 1 - j) * stag)
    )
    assign = [NT - nact_of(c) + c % nact_of(c) for c in range(nch)]
    first_c = {}
    last_c = {}
    for c, j in enumerate(assign):
        first_c.setdefault(j, c)
        last_c[j] = c
    jmin = min(last_c)
    nres = len(last_c)
    nc = bacc.Bacc("TRN2", target_bir_lowering=False)
    x_d = nc.dram_tensor("x", [128, J * F], dtype, kind="ExternalInput")
    ones_d = nc.dram_tensor("ones", [128, MO], dtype, kind="ExternalInput")
    out_d = nc.dram_tensor("out", [nres, 512], mybir.dt.float32, kind="ExternalOutput")

    with TileContext(nc) as tc:
        with (
            tc.tile_pool(name="wpool", bufs=1) as wpool,
            tc.tile_pool(name="xpool", bufs=XBUFS) as xpool,
            tc.tile_pool(name="ppool", bufs=1, space="PSUM") as ppool,
            tc.tile_pool(name="opool", bufs=1) as opool,
        ):
            ones_sb = wpool.tile([128, MO], dtype)
            (nc.scalar if TWO_Q else nc.sync).dma_start(out=ones_sb, in_=ones_d[:, :])
            psum = ppool.tile([128, 512], mybir.dt.float32)
            out_sb = opool.tile([128, 512], mybir.dt.float32)
            ncopy = 0
            for d, (pos, sd) in enumerate(plan):
                xt = xpool.tile([128, SD * F], dtype)
                src = x_d[:, pos * F : (pos + sd) * F]
                if SPLIT_DMA:
                    nc.sync.dma_start(out=xt[:64, : sd * F], in_=src[:64, :])
                    nc.scalar.dma_start(out=xt[64:, : sd * F], in_=src[64:, :])
                else:
                    eng = nc.scalar if (TWO_Q and d % 2) else nc.sync
                    eng.dma_start(out=xt[:, : sd * F], in_=src)
                for u in range(sd // CSLOTS):
                    c = pos // CSLOTS + u
                    j = assign[c]
                    o = COL_OFF[j]
                    nc.tensor.matmul(
                        psum[o : o + MO, :],
                        ones_sb[:, :],
                        xt[:, u * CH : (u + 1) * CH],
                        start=(c == first_c[j]),
                        stop=(c == last_c[j]),
                        tile_position=(0, o),
                    )
            # PSUM evacuation split across DVE + ACT so the tail copies run
            # in parallel instead of serializing behind DVE drains (DVE
            # cannot take partition-strided APs, so one copy per quadrant).
            for i, j in enumerate(sorted(last_c)):
                o = COL_OFF[j]
                if i % 2 == 0:
                    nc.vector.tensor_copy(out_sb[o : o + 1, :], psum[o : o + 1, :])
                else:
                    nc.scalar.copy(out_sb[o : o + 1, :], psum[o : o + 1, :])
            lo = COL_OFF[jmin]
            nc.sync.dma_start(
                out=out_d[:, :], in_=out_sb[lo : lo + 32 * nres : 32, :]
            )
    nc.compile()
    return nc


def _get_bass(J: int) -> bass.Bass:
    key = (J, SD, XBUFS, TWO_Q, SPLIT_DMA, NT, WARM, WARM_SD, COOL, COOL_SD, STAG)
    if key not in _bass_cache:
        _bass_cache[key] = _build_bass(J)
    return _bass_cache[key]


def _quantize_ns(xs: np.ndarray) -> np.ndarray:
    """Noise-shaped e4m3 quantization of xs [n, F] (fp32, pre-scaled).

    Rows are chained in runs of CHAIN_L: q_i = Q(x_i + c_i),
    c_{i+1} = x_i + c_i - q_i.  Within a chain the quantization error
    telescopes, so any full-chain sum is exact to one final carry.
    """
    n = xs.shape[0]
    K = n // CHAIN_L
    q = np.empty((n, F), E4M3)
    if K:
        v = xs[: K * CHAIN_L].reshape(K, CHAIN_L, F)
        qv = q[: K * CHAIN_L].reshape(K, CHAIN_L, F)
        c = np.zeros((K, F), np.float32)
        for j in range(CHAIN_L):
            t = v[:, j, :] + c
            qj = t.astype(E4M3)
            qv[:, j, :] = qj
            c = t - qj.astype(np.float32)
    q[K * CHAIN_L :] = xs[K * CHAIN_L :].astype(E4M3)
    return q


def _run(q: np.ndarray, trace: bool = False, tmpdir=None):
    """Shard quantized rows q [n, 64] (e4m3) over 8 cores, return
    (row-sum [64] as float64 in alpha-scaled units, BassKernelResults)."""
    n = q.shape[0]
    # per-core rows: multiple of 128 partitions * CSLOTS chunk granularity
    # (only the last core ever sees zero-padding)
    nloc = -(-n // NC)
    nloc = -(-nloc // (128 * CSLOTS)) * (128 * CSLOTS)
    J = nloc // 128

    ones = np.ones((128, MO), E4M3)
    in_maps = []
    for c in range(NC):
        lo, hi = c * nloc, (c + 1) * nloc
        if hi <= n:
            qc = q[lo:hi]
        else:
            qc = np.zeros((nloc, F), E4M3)
            if lo < n:
                qc[: n - lo] = q[lo:n]
        in_maps.append({"x": qc.reshape(128, J * F), "ones": ones})

    nc = _get_bass(J)
    res = run_bass_kernel_spmd(
        nc, in_maps, core_ids=list(range(NC)), trace=trace, tmpdir=tmpdir
    )
    total = np.zeros(F, np.float64)
    for c in range(NC):
        o = np.asarray(res.results[c]["out"], np.float64)  # [NT, 512]
        total += o.reshape(-1, F).sum(axis=0)
    return total, res


def kernel(x_atom_fea, segment_ids, num_segments=None, **_ignored):
    x = np.asarray(x_atom_fea, dtype=np.float32)
    seg = np.asarray(segment_ids).astype(np.int64, copy=False)
    n0 = int(num_segments) if num_segments is not None else N0_DEFAULT
    counts = np.bincount(seg, minlength=n0)
    wlut = (1.0 / np.maximum(counts, 1)).astype(np.float32)
    xs = x * wlut[seg][:, None]  # fold per-row weight
    # power-of-2 scale keeps the dequant exact and the e4m3 stream well
    # inside normal range (max 448; leave headroom for feedback carries)
    m = float(np.abs(xs).max())
    alpha = float(2.0 ** np.floor(np.log2(240.0 / m))) if m > 0 else 1.0
    np.multiply(xs, np.float32(alpha), out=xs)
    q = _quantize_ns(xs)
    total, _ = _run(q)
    return (total / (alpha * float(n0))).astype(np.float32).reshape(1, F)


# revision 31
# speedup vs baseline: 1.1281x; 1.1281x over previous
"""Segment-mean-of-means kernel for Trainium2 (8 NeuronCores, SPMD).

Problem: out = mean_s( segment_sum(x)[s] / max(count_s, 1) ) over 65536
segments of a [4M, 64] fp32 tensor with *sorted* segment ids.

Mathematical reformulation: every atom i in segment s contributes
x_i / count_s to the segment mean, so

    out[f] = (1/N0) * sum_s segsum_s[f]/count_s = (1/N0) * sum_i w_i * x_i[f]

with per-row weight w_i = 1 / count_{seg(i)}.  Empty segments contribute
nothing, exactly matching the reference's max(count,1) clamp.

Device kernel = pure streaming row-sum in FP8 (e4m3):
  - host: w folded into x (x' = w*x, scaled by a power-of-2 alpha), then
    *noise-shaped* quantization to e4m3: rows are processed in chains of
    L consecutive rows; each element absorbs the previous element's
    quantization error (error feedback / sigma-delta).  The chain's total
    error telescopes to a single final carry, so the *sum* of the fp8
    stream matches the fp64 sum to ~1e-3 relative even though individual
    elements only carry 3 mantissa bits.  (Plain RTN fp8 fails: 3e-2.)
  - device (per core, 1/8 of rows): PSUM-accumulated PE matmuls with an
    all-ones fp8 lhsT, using COLUMN TILING: the 128x128 PE array is split
    into NT independent 128x32 column tiles, each with its own XBUS ifmap
    stream, so NT matmuls run concurrently -> NT fp8 elements per
    partition-cycle (the single-stream ifmap rate, 1 elem/cycle/partition
    @2.4GHz, is otherwise the bottleneck: 104us/core).
  - host: sum the tiny per-core partials, divide by alpha*N0.

Layout: each core's nloc rows are reshaped [128, J, 64] -- partition k
owns rows [k*J, (k+1)*J), a J*64-byte CONTIGUOUS stream in DRAM.  A DMA
moves SD slots for all partitions (one contiguous SD*64-byte descriptor
per partition; big descriptors keep the HWDGE rings off the critical
path).  The PE consumes 512-byte chunks (8 slots):
  chunk c: lhsT = ones[128, 16] @ column quadrant 32*(c%NT),
           rhs  = chunk [128, 512]
  -> psum[32*(c%NT) .. +16 rows (identical), 0:512]
accumulated over that tile's chunks (start on first, stop on last).
psum column n holds a partial of feature (n % 64); the host folds the
8-slot blocks, quadrants, and cores.
"""

import os

import numpy as np

import concourse.bass as bass
import concourse.mybir as mybir
from concourse import bacc
from concourse.bass_utils import run_bass_kernel_spmd
from concourse.tile import TileContext

import ml_dtypes

E4M3 = np.dtype(ml_dtypes.float8_e4m3fn)


def _harden_trace_path():
    """If a caller enables tracing (e.g. BASS_TRACE=1), run_bass_kernel_spmd
    imports antenv.axon_hooks, which this image lacks -- that would crash the
    run.  Provide the hook via trn_boot's ctypes shim (or a None hook, which
    bass_utils degrades on gracefully), and make the artifact upload failure
    non-fatal (zero-egress sandbox)."""
    import sys
    import types

    try:
        import antenv.axon_hooks  # noqa: F401  # already provided: nothing to do
        return
    except ImportError:
        pass
    hook = None
    try:
        import trn_agent_boot.trn_boot as tb

        hook = tb._ntff_profile_via_ctypes("/opt/axon/libaxon_pjrt.so")
    except Exception:
        pass
    mod = types.ModuleType("antenv.axon_hooks")
    mod.get_axon_ntff_profile_hook = lambda: hook
    sys.modules["antenv.axon_hooks"] = mod

    import concourse.bass_utils as bu

    _orig_upload = bu.upload_artifacts

    def _safe_upload(tmpdir):
        try:
            return _orig_upload(tmpdir)
        except Exception:
            return tmpdir

    bu.upload_artifacts = _safe_upload


_harden_trace_path()

F = 64  # features
NC = 8  # cores
SD = int(os.environ.get("KERNEL_SD", "256"))  # slots per DMA (SD*64 B/partition)
WARM = int(os.environ.get("KERNEL_WARM", "4"))  # small leading DMAs
WARM_SD = int(os.environ.get("KERNEL_WARM_SD", "64"))  # slots per warmup DMA
COOL = int(os.environ.get("KERNEL_COOL", "2"))  # small trailing DMAs
COOL_SD = int(os.environ.get("KERNEL_COOL_SD", "32"))  # slots per cooldown DMA
STAG = int(os.environ.get("KERNEL_STAG", "8"))  # stop-stagger, chunks/quadrant
XBUFS = int(os.environ.get("KERNEL_XBUFS", "10"))  # x tile buffering depth
TWO_Q = os.environ.get("KERNEL_2Q", "1") == "1"  # alternate SP/Act HWDGE rings
SPLIT_DMA = os.environ.get("KERNEL_SPLIT", "0") == "1"  # split partitions across rings
NT = int(os.environ.get("KERNEL_NT", "4"))  # concurrent PE column tiles (1..4)
CHAIN_L = int(os.environ.get("KERNEL_L", "256"))  # noise-shaping chain length
N0_DEFAULT = 65536

CH = 512  # rhs bytes/partition per matmul chunk (psum free = 512 fp32)
CSLOTS = CH // F  # 8 slots per chunk
MO = 16  # lhsT (all-ones) columns -> psum rows, identical; col_grp rounds to 32
COL_OFF = {1: [0], 2: [0, 64], 3: [0, 32, 64], 4: [0, 32, 64, 96]}[NT]
assert SD % CSLOTS == 0

_bass_cache: dict = {}


def _build_bass(J: int) -> bass.Bass:
    """One-core SPMD program: fp8 row-sum of 128*J rows ([128, J*64] layout)."""
    dtype = mybir.dt.float8e4
    nch = J // CSLOTS  # total matmul chunks
    # DMA plan: WARM small tiles first (early PE start), SD-slot tiles, then
    # COOL small tiles (short post-last-data matmul tail)
    plan = []
    pos = 0
    cool_slots = min(COOL * COOL_SD, J)
    while pos < J:
        left = J - pos
        if len(plan) < WARM:
            sd = WARM_SD
        elif left > cool_slots:
            sd = min(SD, left - cool_slots)
        else:
            sd = COOL_SD
        sd = min(sd, left)
        plan.append((pos, sd))
        pos += sd
    # chunk -> column-tile assignment: round-robin, but quadrant j goes
    # quiet STAG*(NT-1-j) chunks before the end so its PSUM evacuation
    # overlaps the remaining stream instead of serializing after it.
    stag = STAG if nch >= NT * STAG * 4 else 0
    nact_of = lambda c: max(
        1, sum(1 for j in range(NT) if c < nch - (NT - 1 - j) * stag)
    )
    assign = [NT - nact_of(c) + c % nact_of(c) for c in range(nch)]
    first_c = {}
    last_c = {}
    for c, j in enumerate(assign):
        first_c.setdefault(j, c)
        last_c[j] = c
    jmin = min(last_c)
    nres = len(last_c)
    nc = bacc.Bacc("TRN2", target_bir_lowering=False)
    x_d = nc.dram_tensor("x", [128, J * F], dtype, kind="ExternalInput")
    ones_d = nc.dram_tensor("ones", [128, MO], dtype, kind="ExternalInput")
    out_d = nc.dram_tensor("out", [nres, 512], mybir.dt.float32, kind="ExternalOutput")

    with TileContext(nc) as tc:
        with (
            tc.tile_pool(name="wpool", bufs=1) as wpool,
            tc.tile_pool(name="xpool", bufs=XBUFS) as xpool,
            tc.tile_pool(name="ppool", bufs=1, space="PSUM") as ppool,
            tc.tile_pool(name="opool", bufs=1) as opool,
        ):
            ones_sb = wpool.tile([128, MO], dtype)
            (nc.scalar if TWO_Q else nc.sync).dma_start(out=ones_sb, in_=ones_d[:, :])
            psum = ppool.tile([128, 512], mybir.dt.float32)
            out_sb = opool.tile([128, 512], mybir.dt.float32)
            ncopy = 0
            for d, (pos, sd) in enumerate(plan):
                xt = xpool.tile([128, SD * F], dtype)
                src = x_d[:, pos * F : (pos + sd) * F]
                if SPLIT_DMA:
                    nc.sync.dma_start(out=xt[:64, : sd * F], in_=src[:64, :])
                    nc.scalar.dma_start(out=xt[64:, : sd * F], in_=src[64:, :])
                else:
                    eng = nc.scalar if (TWO_Q and d % 2) else nc.sync
                    eng.dma_start(out=xt[:, : sd * F], in_=src)
                for u in range(sd // CSLOTS):
                    c = pos // CSLOTS + u
                    j = assign[c]
                    o = COL_OFF[j]
                    nc.tensor.matmul(
                        psum[o : o + MO, :],
                        ones_sb[:, :],
                        xt[:, u * CH : (u + 1) * CH],
                        start=(c == first_c[j]),
                        stop=(c == last_c[j]),
                        tile_position=(0, o),
                    )
            # PSUM evacuation: per-quadrant DVE copy, each immediately
            # followed by its own contiguous out DMA.  (A single
            # partition-strided gather DMA after engine-split copies was
            # observed to race intermittently; region-exact copy->DMA pairs
            # on one engine are the conservative, proven pattern.)
            for i, j in enumerate(sorted(last_c)):
                o = COL_OFF[j]
                nc.vector.tensor_copy(out_sb[o : o + 1, :], psum[o : o + 1, :])
                nc.sync.dma_start(out=out_d[i : i + 1, :], in_=out_sb[o : o + 1, :])
    nc.compile()
    return nc


def _get_bass(J: int) -> bass.Bass:
    key = (J, SD, XBUFS, TWO_Q, SPLIT_DMA, NT, WARM, WARM_SD, COOL, COOL_SD, STAG)
    if key not in _bass_cache:
        _bass_cache[key] = _build_bass(J)
    return _bass_cache[key]


def _quantize_ns(xs: np.ndarray) -> np.ndarray:
    """Noise-shaped e4m3 quantization of xs [n, F] (fp32, pre-scaled).

    Rows are chained in runs of CHAIN_L: q_i = Q(x_i + c_i),
    c_{i+1} = x_i + c_i - q_i.  Within a chain the quantization error
    telescopes, so any full-chain sum is exact to one final carry.
    """
    n = xs.shape[0]
    K = n // CHAIN_L
    q = np.empty((n, F), E4M3)
    if K:
        v = xs[: K * CHAIN_L].reshape(K, CHAIN_L, F)
        qv = q[: K * CHAIN_L].reshape(K, CHAIN_L, F)
        c = np.zeros((K, F), np.float32)
        for j in range(CHAIN_L):
            t = v[:, j, :] + c
            qj = t.astype(E4M3)
            qv[:, j, :] = qj
            c = t - qj.astype(np.float32)
    q[K * CHAIN_L :] = xs[K * CHAIN_L :].astype(E4M3)
    return q


def _run(q: np.ndarray, trace: bool = False, tmpdir=None):
    """Shard quantized rows q [n, 64] (e4m3) over 8 cores, return
    (row-sum [64] as float64 in alpha-scaled units, BassKernelResults)."""
    n = q.shape[0]
    # per-core rows: multiple of 128 partitions * CSLOTS chunk granularity
    # (only the last core ever sees zero-padding)
    nloc = -(-n // NC)
    nloc = -(-nloc // (128 * CSLOTS)) * (128 * CSLOTS)
    J = nloc // 128

    ones = np.ones((128, MO), E4M3)
    in_maps = []
    for c in range(NC):
        lo, hi = c * nloc, (c + 1) * nloc
        if hi <= n:
            qc = q[lo:hi]
        else:
            qc = np.zeros((nloc, F), E4M3)
            if lo < n:
                qc[: n - lo] = q[lo:n]
        in_maps.append({"x": qc.reshape(128, J * F), "ones": ones})

    nc = _get_bass(J)
    res = run_bass_kernel_spmd(
        nc, in_maps, core_ids=list(range(NC)), trace=trace, tmpdir=tmpdir
    )
    total = np.zeros(F, np.float64)
    for c in range(NC):
        o = np.asarray(res.results[c]["out"], np.float64)  # [NT, 512]
        total += o.reshape(-1, F).sum(axis=0)
    return total, res


def kernel(x_atom_fea, segment_ids, num_segments=None, **_ignored):
    x = np.asarray(x_atom_fea, dtype=np.float32)
    seg = np.asarray(segment_ids).astype(np.int64, copy=False)
    n0 = int(num_segments) if num_segments is not None else N0_DEFAULT
    counts = np.bincount(seg, minlength=n0)
    wlut = (1.0 / np.maximum(counts, 1)).astype(np.float32)
    xs = x * wlut[seg][:, None]  # fold per-row weight
    # power-of-2 scale keeps the dequant exact and the e4m3 stream well
    # inside normal range (max 448; leave headroom for feedback carries)
    m = float(np.abs(xs).max())
    alpha = float(2.0 ** np.floor(np.log2(240.0 / m))) if m > 0 else 1.0
    np.multiply(xs, np.float32(alpha), out=xs)
    q = _quantize_ns(xs)
    total, _ = _run(q)
    return (total / (alpha * float(n0))).astype(np.float32).reshape(1, F)
